# revision 1
# baseline (speedup 1.0000x reference)
"""Trainium2 Bass kernel for multi-head attention (B=4, N=2048, DIM=1024, H=16, DH=64).

Sharding (head-parallel + row-parallel to_out): 8 cores = 4 batches x 2 head-halves.
Each core computes q/k/v for its 8 heads over the FULL 2048-token sequence (no
duplicated projection work, unlike query-split sharding), runs attention for those
heads, and its row-parallel half of the output projection. The to_out all-reduce
happens on the host at gather time: out[b] = partial[core 2b] + partial[core 2b+1]
(bias is added on the even core only; odd cores receive a zero bias input).

The attention phase is ACT(exp)-bound (~1147 ns per [128,1024] exp vs ~860 ns PE
work per step), so projections are hoisted into a DMA-racing pre-phase and the
output projection into dense PE blocks (one mid-phase, one two-pass tail that
overlaps the final softmax-normalize chain). Softmax denominators are folded
into the AV matmul via a ones-column in V; the reciprocal is spread over 64
partitions via an SBUF DMA restructure (293 ns vs 6.5 us single-partition), then
broadcast back through a DRAM round-trip.

Trainium2's PE clock gate (HAM) is sticky: one ~3.4 us idle window drops the PE
to 1.2 GHz, and the ACT-bound attention steps never present the sustained-busy
window needed to recover -- which would cost ~100 us per incident. Redundant
"warm-guard" matmul bursts (alternating row groups so they pipeline
back-to-back, disjoint PSUM halves so they may run concurrently) at kernel
start and at every attention block entry bound any cold excursion to one block.
"""

import numpy as np
import ml_dtypes

import concourse.bass as bass
import concourse.tile as tile
from concourse import bacc, mybir
from concourse import bass_utils

B, N, DIM = 4, 2048, 1024
HEADS_TOT, DH = 16, 64
SCALE = DH ** -0.5
NCORES = 8

HPC = 8              # heads per core
NPAIR = HPC // 2     # head-pairs per core = 4
KT = DIM // 128      # 8 contraction tiles
NT = N // 128        # 16 key tiles
IC = 2               # query blocks
IB = N // IC         # 1024 queries per block
BF16 = mybir.dt.bfloat16
F32 = mybir.dt.float32

_CACHE = {}


def _build_program():
    nc = bacc.Bacc("TRN2", target_bir_lowering=False, debug=False)

    xT_d = nc.dram_tensor("xT", [128, KT, N], BF16, kind="ExternalInput")
    w_d = nc.dram_tensor("w_qkv", [128, 3, KT, 512], BF16, kind="ExternalInput")
    wo_d = nc.dram_tensor("w_out", [128, NPAIR, DIM], BF16, kind="ExternalInput")
    out_d = nc.dram_tensor("out", [N, DIM], BF16, kind="ExternalOutput")

    with tile.TileContext(nc) as tc:
        _emit(tc, nc, xT_d, w_d, wo_d, out_d)
    nc.compile()
    return nc


def _emit(tc, nc, xT_d, w_d, wo_d, out_d):
    from contextlib import ExitStack

    with ExitStack() as ctx:
        consts = ctx.enter_context(tc.tile_pool(name="consts", bufs=1))
        stage = ctx.enter_context(tc.tile_pool(name="stage", bufs=1))
        qkv = ctx.enter_context(tc.tile_pool(name="qkv", bufs=1))
        ao = ctx.enter_context(tc.tile_pool(name="ao", bufs=1))
        atp = ctx.enter_context(tc.tile_pool(name="atp", bufs=8))
        avup = ctx.enter_context(tc.tile_pool(name="avu", bufs=2))
        rcp = ctx.enter_context(tc.tile_pool(name="rcp", bufs=2))
        bcsp = ctx.enter_context(tc.tile_pool(name="bcs", bufs=2))
        oddp = ctx.enter_context(tc.tile_pool(name="odd", bufs=2))
        drbp = ctx.enter_context(tc.tile_pool(name="drb", bufs=2, space="DRAM"))
        stp = ctx.enter_context(tc.tile_pool(name="stp", bufs=2))
        ppp = ctx.enter_context(tc.tile_pool(name="ppp", bufs=1))

        # ---- constants / weights DMA (wv first: it gates the first matmuls) ----
        wo_sb = consts.tile([128, NPAIR, DIM], BF16)
        at_warm = consts.tile([128, 32], BF16)
        wv = stage.tile([128, KT, 512], BF16)
        wk = stage.tile([128, KT, 512], BF16)
        wq = stage.tile([128, KT, 512], BF16)
        for kt in range(KT):
            nc.scalar.dma_start(out=wv[:, kt, :], in_=w_d.ap()[:, 0, kt, :])

        # preload exp table set early (off the critical path)
        nc.scalar.activation(out=at_warm, in_=wv[:, 0, 0:32],
                             func=mybir.ActivationFunctionType.Exp)

        nc.sync.dma_start(out=wk, in_=w_d.ap()[:, 1])
        nc.gpsimd.dma_start(out=wq, in_=w_d.ap()[:, 2])
        nc.scalar.dma_start(out=wo_sb, in_=wo_d.ap())

        xTk = [stage.tile([128, N], BF16, name=f"xTk{k}") for k in range(KT)]
        for k in range(KT):
            eng = (nc.sync, nc.gpsimd, nc.scalar)[k % 3]
            eng.dma_start(out=xTk[k], in_=xT_d.ap()[:, k, :])

        # ---- persistent qkv / attention-out tiles ----
        kTs = [qkv.tile([128, N], BF16, name=f"kT{s}") for s in range(NPAIR)]
        qTs = [qkv.tile([128, N], BF16, name=f"qT{s}") for s in range(NPAIR)]
        vt = qkv.tile([128, NT, HPC, DH + 1], BF16)
        nc.vector.memset(vt[:, :, :, DH], 1.0)
        aoTs = [ao.tile([128, N], BF16, name=f"aoT{s}") for s in range(NPAIR)]

        # ---- phase 1: all projections, racing the input DMAs ----
        with tc.tile_pool(name="pre_ps", bufs=4, space="PSUM") as prep:
            def pre_pool():
                return (prep, "p")

            def vchunk(t):
                pool, tag = pre_pool()
                ps = pool.tile([128, 512], F32, tag=tag, name=f"vps{t}")
                for kt in range(KT):
                    nc.tensor.matmul(ps, xTk[kt][:, 128 * t:128 * (t + 1)],
                                     wv[:, kt, :],
                                     start=(kt == 0), stop=(kt == KT - 1))
                nc.vector.tensor_copy(
                    out=vt[:, t, :, 0:DH],
                    in_=ps.rearrange("p (h d) -> p h d", h=HPC))

            def kqchunk(w, dst, s, c):
                pool, tag = pre_pool()
                ps = pool.tile([128, 512], F32, tag=tag, name=f"ps{s}_{c}")
                for kt in range(KT):
                    nc.tensor.matmul(ps, w[:, kt, 128 * s:128 * (s + 1)],
                                     xTk[kt][:, 512 * c:512 * (c + 1)],
                                     start=(kt == 0), stop=(kt == KT - 1))
                nc.vector.tensor_copy(out=dst[s][:, 512 * c:512 * (c + 1)], in_=ps)

            # instant HAM warm-up: garbage MMs on uninitialized SBUF warm the
            # PE clock-gate (~6.8us alternating row groups) while DMAs land
            ps_w0 = prep.tile([128, 512], F32, tag="p", name="ps_w0")
            ps_w1 = prep.tile([128, 512], F32, tag="p", name="ps_w1")
            for i in range(16):
                wb = 64 * (i % 2)
                nc.tensor.matmul(ps_w0 if wb == 0 else ps_w1,
                                 kTs[0][wb:wb + 64, 0:128],
                                 kTs[0][wb:wb + 64, 0:512],
                                 start=True, stop=True, tile_position=(wb, 0))

            for t in range(NT):
                vchunk(t)
            for s in range(NPAIR):
                for c in range(4):
                    kqchunk(wk, kTs, s, c)
            for s in range(NPAIR):
                for c in range(4):
                    kqchunk(wq, qTs, s, c)

        scp = ctx.enter_context(tc.tile_pool(name="scp", bufs=2, space="PSUM"))
        avp = ctx.enter_context(tc.tile_pool(name="avp", bufs=2, space="PSUM"))

        def normalize(s, ic, p, av):
            sfx = f"{s}_{ic}_{p}"
            avu = avup.tile([DH + 1, IB], F32, tag="avu", name=f"avu{sfx}")
            nc.vector.tensor_copy(out=avu, in_=av)
            # spread the denominator over 64 partitions so the reciprocal runs
            # 64-wide (SBUF->SBUF DMA partition restructure) instead of 6.5us
            # on a single partition
            dsp = rcp.tile([DH, IB // DH], F32, tag="dsp", name=f"dsp{sfx}")
            nc.sync.dma_start(out=dsp, in_=avu[DH:DH + 1, :])
            rc = rcp.tile([DH, IB // DH], F32, tag="rc", name=f"rc{sfx}")
            nc.vector.reciprocal(out=rc, in_=dsp)
            dr = drbp.tile([IB], F32, tag="dr", name=f"dr{sfx}")
            dr_sq = bass.AP(tensor=dr.tensor, offset=dr.offset,
                            ap=[[IB // DH, DH], [1, IB // DH]])
            nc.sync.dma_start(out=dr_sq, in_=rc)
            dr_bc = bass.AP(tensor=dr.tensor, offset=dr.offset,
                            ap=[[0, DH]] + [list(dd) for dd in dr.ap])
            bcs = bcsp.tile([DH, IB], F32, tag="bcs", name=f"bcs{sfx}")
            nc.sync.dma_start(out=bcs, in_=dr_bc)
            if p == 0:
                with nc.allow_low_precision(reason="attn out in bf16"):
                    nc.vector.tensor_mul(
                        out=aoTs[s][0:DH, IB * ic:IB * (ic + 1)],
                        in0=avu[0:DH, :], in1=bcs)
            else:
                od = oddp.tile([DH, IB], BF16, tag="od", name=f"od{s}_{ic}")
                with nc.allow_low_precision(reason="attn out in bf16"):
                    nc.vector.tensor_mul(out=od, in0=avu[0:DH, :], in1=bcs)
                nc.gpsimd.dma_start(
                    out=aoTs[s][DH:128, IB * ic:IB * (ic + 1)], in_=od)

        def block(s, ic, warm=False):
            av0 = avp.tile([DH + 1, IB], F32, tag="av", name=f"av0_{s}_{ic}")
            av1 = avp.tile([DH + 1, IB], F32, tag="av", name=f"av1_{s}_{ic}")
            avs = [av0, av1]
            pend = [None]

            def emit_av(t, p, at):
                h = 2 * s + p
                for c in range(2):
                    nc.tensor.matmul(
                        avs[p][:, 512 * c:512 * (c + 1)],
                        vt[:, t, h, :],
                        at[:, 512 * c:512 * (c + 1)],
                        start=(t == 0), stop=(t == NT - 1))

            for t in range(NT):
                for p in range(2):
                    pb = 64 * p
                    sc = scp.tile([128, IB], F32, tag="sc", name=f"sc{s}_{ic}_{t}_{p}")
                    nburst = 16 if (warm == 2 and t == 0 and p == 0) else \
                        (8 if (warm and t == 0 and p == 0) else 0)
                    if nburst:
                        # HAM warm-guard: 16 redundant score MMs alternating row
                        # groups (h0/h64 pipeline back-to-back) give the PE a
                        # ~6.8us contiguous-busy run when cold -- guaranteed to
                        # cover a full aligned HAM SHORT window, so a cold
                        # (K=4/8) clock-gate state cannot outlive a block.
                        # Their (garbage) output is overwritten by the real MMs.
                        for i in range(nburst):
                            wb = 64 * (i % 2)
                            nc.tensor.matmul(
                                sc[:, 512 * (i % 2):512 * (i % 2) + 512],
                                kTs[s][wb:wb + 64, 0:128],
                                qTs[s][wb:wb + 64, IB * ic:IB * ic + 512],
                                start=True, stop=True, tile_position=(wb, 0))
                    for c in range(2):
                        nc.tensor.matmul(
                            sc[:, 512 * c:512 * (c + 1)],
                            kTs[s][pb:pb + 64, 128 * t:128 * (t + 1)],
                            qTs[s][pb:pb + 64,
                                   IB * ic + 512 * c:IB * ic + 512 * (c + 1)],
                            start=True, stop=True, tile_position=(pb, 0))
                    at = atp.tile([128, IB], BF16, tag="at",
                                  name=f"at{s}_{ic}_{t}_{p}")
                    nc.scalar.activation(out=at, in_=sc,
                                         func=mybir.ActivationFunctionType.Exp,
                                         scale=SCALE)
                    # lag the AV matmuls one step: the PE runs the previous
                    # step's AV during this step's exp instead of stalling
                    if pend[0] is not None:
                        emit_av(*pend[0])
                    pend[0] = (t, p, at)
            emit_av(*pend[0])
            pend[0] = None
            for p in (1, 0):
                normalize(s, ic, p, avs[p])

        def oproj(ns):
            po = scp.tile([128, DIM], F32, tag="sc", name=f"po{ns}")
            for c in range(2):
                for hp in range(NPAIR):
                    nc.tensor.matmul(
                        po[:, 512 * c:512 * (c + 1)],
                        aoTs[hp][:, 128 * ns:128 * (ns + 1)],
                        wo_sb[:, hp, 512 * c:512 * (c + 1)],
                        start=(hp == 0), stop=(hp == NPAIR - 1))
            st = stp.tile([128, DIM], BF16, tag="st", name=f"st{ns}")
            nc.vector.tensor_copy(out=st, in_=po)
            eng = (nc.sync, nc.gpsimd, nc.scalar)[ns % 3]
            eng.dma_start(out=out_d.ap()[128 * ns:128 * (ns + 1), :], in_=st)

        # ---- attention + output projection ----
        for s in range(NPAIR):
            block(s, 0, warm=(2 if s == 0 else 1))
        for s in range(NPAIR):
            block(s, 1, warm=1)
        # out-proj for query block 0 runs in the tail shadow of the final
        # softmax-normalize chain (keeps ACT saturated through both phases)
        for ns in range(0, 8):
            oproj(ns)
        # by the time the ic0 rows (above) finish, the final normalize chain has
        # landed, so ic1 rows use uniform single-pass chunks and stream out early
        for ns in range(8, 16):
            oproj(ns)


def get_program():
    if "nc" not in _CACHE:
        _CACHE["nc"] = _build_program()
    return _CACHE["nc"]


def make_in_maps(x, w_qkv, w_out, b_out):
    bf = ml_dtypes.bfloat16
    x = np.asarray(x, np.float32)
    w_qkv = np.asarray(w_qkv, np.float32)
    w_out = np.asarray(w_out, np.float32)
    b_out = np.asarray(b_out, np.float32)

    in_maps = []
    for core in range(NCORES):
        b, hh = core // 2, core % 2
        # xT in [128, KT, N] layout: [p, t, n] = x[b].T[t*128+p, n]
        xT = np.ascontiguousarray(x[b].T).astype(bf)                 # [DIM, N]
        xT_pt = np.ascontiguousarray(xT.reshape(KT, 128, N).transpose(1, 0, 2))
        # w slices for this head-half, groups ordered [v, k, q]
        wq = w_qkv[:, 512 * hh:512 * (hh + 1)]
        wk = w_qkv[:, DIM + 512 * hh:DIM + 512 * (hh + 1)]
        wv = w_qkv[:, 2 * DIM + 512 * hh:2 * DIM + 512 * (hh + 1)]
        wcat = np.stack([wv, wk, wq], axis=0).astype(bf)             # [3, DIM, 512]
        w_pt = np.ascontiguousarray(
            wcat.reshape(3, KT, 128, 512).transpose(2, 0, 1, 3))    # [p, g, t, e]
        # w_out rows for this half -> [p, hp, d]
        wo = w_out[512 * hh:512 * (hh + 1), :].astype(bf)            # [512, DIM]
        wo_pt = np.ascontiguousarray(wo.reshape(NPAIR, 128, DIM).transpose(1, 0, 2))
        in_maps.append({
            "xT": xT_pt,
            "w_qkv": w_pt,
            "w_out": wo_pt,
        })
    return in_maps


def kernel(x, w_qkv, w_out, b_out):
    nc = get_program()
    in_maps = make_in_maps(x, w_qkv, w_out, b_out)
    res = bass_utils.run_bass_kernel_spmd(nc, in_maps, core_ids=list(range(NCORES)))
    out = np.empty((B, N, DIM), np.float32)
    bias = np.asarray(b_out, np.float32)
    for b in range(B):
        out[b] = np.asarray(res.results[2 * b]["out"], np.float32)
        out[b] += np.asarray(res.results[2 * b + 1]["out"], np.float32)
        out[b] += bias
    return out



# revision 7
# speedup vs baseline: 1.1696x; 1.1696x over previous
"""Trainium2 Bass kernel for multi-head attention (B=4, N=2048, DIM=1024, H=16, DH=64).

Sharding (head-parallel + row-parallel to_out): 8 cores = 4 batches x 2 head-halves.
Each core computes q/k/v for its 8 heads over the FULL 2048-token sequence, runs
attention for those heads, and its row-parallel half of the output projection. The
to_out all-reduce happens on the host at gather time:
out[b] = partial[core 2b] + partial[core 2b+1] + bias.

The attention phase is ACT(exp)-bound: softmax exp runs only on the scalar engine at
1 elem/cycle/lane, so the per-core floor is 256 x [128,1024]-exp = ~294 us. This
kernel keeps the ACT stream contiguous from ~18us on by:
  * blocks of (head-pair s, 512-query block qb): both heads' scores live side by
    side in ONE [128,1024] fp32 PSUM tile (2 banks) -> one exp instruction per
    key-tile step covers both heads at full FD=1024 efficiency.
  * score matmuls for the two heads are issued back-to-back into disjoint PE row
    groups (tile_position (0,0)/(64,0)) so they stream concurrently (~2x).
  * av accumulators are [65,512] (1 PSUM bank each, ones-column denominator), so
    PSUM = 4(sc double-buffer) + 2(av) + 2(util) banks, leaving a util pool for
    projection matmuls to interleave with attention.
  * input DMAs are chunked and priority-ordered (first key/query weight slices,
    x column blocks in consumption order), so the first exp fires at ~18us; the
    qkv projections, late q chunks, and the output projection stream into the
    per-step PE slack via a budget-tracked filler queue.
  * a ~44-matmul garbage warm-up burst keeps the PE HAM clock-gate busy across
    the initial DMA wait so projection matmuls run at 2.4 GHz from the start.
Softmax denominators are folded into the AV matmul via a ones-column in V; the
reciprocal is spread over 64 partitions via an SBUF DMA restructure, then
broadcast back through a DRAM round-trip.
"""

import numpy as np
import ml_dtypes

import concourse.bass as bass
import concourse.tile as tile
from concourse import bacc, mybir
from concourse import bass_utils

B, N, DIM = 4, 2048, 1024
HEADS_TOT, DH = 16, 64
SCALE = DH ** -0.5
NCORES = 8

HPC = 8              # heads per core
NPAIR = HPC // 2     # head-pairs per core = 4
KT = DIM // 128      # 8 contraction tiles
NT = N // 128        # 16 key tiles
QB = 512             # queries per block
NQB = N // QB        # 4 query blocks
BF16 = mybir.dt.bfloat16
F32 = mybir.dt.float32

_CACHE = {}


def _build_program():
    nc = bacc.Bacc("TRN2", target_bir_lowering=False, debug=False)

    xT_d = nc.dram_tensor("xT", [128, KT, N], BF16, kind="ExternalInput")
    w_d = nc.dram_tensor("w_qkv", [128, 3, KT, 512], BF16, kind="ExternalInput")
    wo_d = nc.dram_tensor("w_out", [128, NPAIR, DIM], BF16, kind="ExternalInput")
    out_d = nc.dram_tensor("out", [N, DIM], BF16, kind="ExternalOutput")

    with tile.TileContext(nc) as tc:
        _emit(tc, nc, xT_d, w_d, wo_d, out_d)
    nc.compile()
    return nc


def _emit(tc, nc, xT_d, w_d, wo_d, out_d):
    from contextlib import ExitStack

    with ExitStack() as ctx:
        consts = ctx.enter_context(tc.tile_pool(name="consts", bufs=1))
        stage = ctx.enter_context(tc.tile_pool(name="stage", bufs=1))
        qkv = ctx.enter_context(tc.tile_pool(name="qkv", bufs=1))
        ao = ctx.enter_context(tc.tile_pool(name="ao", bufs=1))
        atp = ctx.enter_context(tc.tile_pool(name="atp", bufs=4))
        avup = ctx.enter_context(tc.tile_pool(name="avu", bufs=2))
        rcp = ctx.enter_context(tc.tile_pool(name="rcp", bufs=2))
        bcsp = ctx.enter_context(tc.tile_pool(name="bcs", bufs=2))
        oddp = ctx.enter_context(tc.tile_pool(name="odd", bufs=2))
        drbp = ctx.enter_context(tc.tile_pool(name="drb", bufs=2, space="DRAM"))
        stp = ctx.enter_context(tc.tile_pool(name="stp", bufs=2))
        scp = ctx.enter_context(tc.tile_pool(name="scp", bufs=2, space="PSUM"))
        avp = ctx.enter_context(tc.tile_pool(name="avp", bufs=2, space="PSUM"))
        utilp = ctx.enter_context(tc.tile_pool(name="utl", bufs=2, space="PSUM"))

        # ---- persistent SBUF tiles ----
        wo_sb = consts.tile([128, NPAIR, DIM], BF16)
        at_warm = consts.tile([128, 32], BF16)
        wv = stage.tile([128, KT, 512], BF16)
        wk = stage.tile([128, KT, 512], BF16)
        wq = stage.tile([128, KT, 512], BF16)
        xTk = [stage.tile([128, N], BF16, name=f"xTk{k}") for k in range(KT)]
        kTs = [qkv.tile([128, N], BF16, name=f"kT{s}") for s in range(NPAIR)]
        qTs = [qkv.tile([128, N], BF16, name=f"qT{s}") for s in range(NPAIR)]
        vt = qkv.tile([128, NT, HPC, DH + 1], BF16)
        aoTs = [ao.tile([128, N], BF16, name=f"aoT{s}") for s in range(NPAIR)]

        # ---- exp table preload (reads garbage SBUF; off the data critical path)
        nc.scalar.activation(out=at_warm, in_=kTs[0][:, 0:32],
                             func=mybir.ActivationFunctionType.Exp)

        # ---- input DMAs: chunked + priority-ordered ----
        # sync queue:  wk s0-slice, then x column blocks (even kt), wk rest
        # gpsimd queue: wq s0-slice, wv, x column blocks (odd kt), wq rest
        # scalar queue: wo (needed late)
        nc.sync.dma_start(out=wk[:, :, 0:128], in_=w_d.ap()[:, 1, :, 0:128])
        nc.gpsimd.dma_start(out=wq[:, :, 0:128], in_=w_d.ap()[:, 2, :, 0:128])
        nc.gpsimd.dma_start(out=wv, in_=w_d.ap()[:, 0])
        for c in range(NQB):
            for k in range(KT):
                eng = (nc.sync, nc.gpsimd)[k % 2]
                eng.dma_start(out=xTk[k][:, 512 * c:512 * (c + 1)],
                              in_=xT_d.ap()[:, k, 512 * c:512 * (c + 1)])
        nc.sync.dma_start(out=wk[:, :, 128:512], in_=w_d.ap()[:, 1, :, 128:512])
        nc.gpsimd.dma_start(out=wq[:, :, 128:512], in_=w_d.ap()[:, 2, :, 128:512])
        nc.scalar.dma_start(out=wo_sb, in_=wo_d.ap())

        nc.vector.memset(vt[:, :, :, DH], 1.0)

        # ---- HAM warm-up: garbage MMs (alternating row groups, 2 util slots)
        # keep the PE busy across the ~12us initial DMA wait so the clock-gate
        # stays at 8/8 when the real projections start.
        ps_w0 = utilp.tile([128, 512], F32, tag="u", name="ps_w0")
        ps_w1 = utilp.tile([128, 512], F32, tag="u", name="ps_w1")
        for i in range(44):
            wb = 64 * (i % 2)
            nc.tensor.matmul(ps_w0 if wb == 0 else ps_w1,
                             kTs[0][wb:wb + 64, 0:128],
                             kTs[0][wb:wb + 64, 0:512],
                             start=True, stop=True, tile_position=(wb, 0))

        # ---- projection emitters (PE work chunks; all write via util pool) ----
        def kq_half(w, dst, s, c, half, cell):
            """4 accumulation MMs; both halves share one PSUM tile via `cell`;
            half 1 finishes the group + copies out."""
            if half == 0:
                cell["ps"] = utilp.tile([128, 512], F32, tag="u",
                                        name=f"kq{s}_{c}_{w is wq}")
            ps = cell["ps"]
            for kt in range(4 * half, 4 * half + 4):
                nc.tensor.matmul(ps, w[:, kt, 128 * s:128 * (s + 1)],
                                 xTk[kt][:, 512 * c:512 * (c + 1)],
                                 start=(kt == 0), stop=(kt == KT - 1))
            if half == 1:
                nc.vector.tensor_copy(out=dst[s][:, 512 * c:512 * (c + 1)], in_=ps)

        def kq_chunk(w, dst, s, c):
            cell = {}
            kq_half(w, dst, s, c, 0, cell)
            kq_half(w, dst, s, c, 1, cell)

        def vchunk(t):
            """v projection for token tile t, all 8 heads (512-wide moving)."""
            ps = utilp.tile([128, 512], F32, tag="u", name=f"vps{t}")
            for kt in range(KT):
                nc.tensor.matmul(ps, xTk[kt][:, 128 * t:128 * (t + 1)],
                                 wv[:, kt, :],
                                 start=(kt == 0), stop=(kt == KT - 1))
            nc.vector.tensor_copy(
                out=vt[:, t, :, 0:DH],
                in_=ps.rearrange("p (h d) -> p h d", h=HPC))

        def oproj_half(ns, c):
            """output projection for token rows 128*ns.., output cols 512*c.."""
            po = utilp.tile([128, 512], F32, tag="u", name=f"po{ns}_{c}")
            for hp in range(NPAIR):
                nc.tensor.matmul(
                    po, aoTs[hp][:, 128 * ns:128 * (ns + 1)],
                    wo_sb[:, hp, 512 * c:512 * (c + 1)],
                    start=(hp == 0), stop=(hp == NPAIR - 1))
            st = stp.tile([128, 512], BF16, tag="st", name=f"st{ns}_{c}")
            nc.vector.tensor_copy(out=st, in_=po)
            eng = (nc.sync, nc.gpsimd)[(2 * ns + c) % 2]
            eng.dma_start(
                out=out_d.ap()[128 * ns:128 * (ns + 1), 512 * c:512 * (c + 1)],
                in_=st)

        # ---- filler queue: (cost_ns, deadline_block, emit_fn) ----
        # deadline_block = block index before which the item MUST be emitted.
        fillers = []

        def push_kq(w, dst, s, c, deadline):
            cell = {}
            fillers.append([900, deadline,
                            lambda: kq_half(w, dst, s, c, 0, cell)])
            fillers.append([900, deadline,
                            lambda: kq_half(w, dst, s, c, 1, cell)])

        # q chunks c=1..3 for s=0 are needed by blocks 1,2,3
        for c in range(1, NQB):
            push_kq(wq, qTs, 0, c, c)
        # kq for s=1..3: k c0..3 + q c0 before block 4*s; later q chunks staggered
        for s in range(1, NPAIR):
            for c in range(NQB):
                push_kq(wk, kTs, s, c, 4 * s)
            push_kq(wq, qTs, s, 0, 4 * s)
            for c in range(1, NQB):
                push_kq(wq, qTs, s, c, 4 * s + c)
        # oproj items are appended dynamically once block (3, qb) completes.

        budget = [0.0]
        STEP_COST = 800.0
        ACT_STEP = 1147.0

        def pop_fillers(blk_idx):
            # force-pop overdue items, then spend any positive budget
            while fillers and fillers[0][1] <= blk_idx:
                cost, _, fn = fillers.pop(0)
                fn()
                budget[0] -= cost
            while fillers and budget[0] >= fillers[0][0]:
                cost, _, fn = fillers.pop(0)
                fn()
                budget[0] -= cost

        # ---- attention block ----
        def block(s, qb, blk_idx, jit_v=False):
            # any filler this block depends on (k/q chunks) MUST be emitted
            # before the block's first score matmul, or the PE FIFO deadlocks
            budget[0] = max(budget[0], 0.0)
            pop_fillers(blk_idx)
            h0, h1 = 2 * s, 2 * s + 1
            av0 = avp.tile([DH + 1, QB], F32, tag="av", name=f"av0_{s}_{qb}")
            av1 = avp.tile([DH + 1, QB], F32, tag="av", name=f"av1_{s}_{qb}")
            pend = [None]

            def emit_av(t, at):
                nc.tensor.matmul(av0, vt[:, t, h0, :], at[:, 0:QB],
                                 start=(t == 0), stop=(t == NT - 1))
                nc.tensor.matmul(av1, vt[:, t, h1, :], at[:, QB:2 * QB],
                                 start=(t == 0), stop=(t == NT - 1))

            for t in range(NT):
                sc = scp.tile([128, 2 * QB], F32, tag="sc",
                              name=f"sc{s}_{qb}_{t}")
                # both heads' scores back-to-back -> disjoint row groups run
                # concurrently on the PE
                nc.tensor.matmul(sc[:, 0:QB],
                                 kTs[s][0:64, 128 * t:128 * (t + 1)],
                                 qTs[s][0:64, QB * qb:QB * (qb + 1)],
                                 start=True, stop=True, tile_position=(0, 0))
                nc.tensor.matmul(sc[:, QB:2 * QB],
                                 kTs[s][64:128, 128 * t:128 * (t + 1)],
                                 qTs[s][64:128, QB * qb:QB * (qb + 1)],
                                 start=True, stop=True, tile_position=(64, 0))
                at = atp.tile([128, 2 * QB], BF16, tag="at",
                              name=f"at{s}_{qb}_{t}")
                nc.scalar.activation(out=at, in_=sc,
                                     func=mybir.ActivationFunctionType.Exp,
                                     scale=SCALE)
                if pend[0] is not None:
                    emit_av(*pend[0])
                pend[0] = (t, at)
                if jit_v:
                    # v projection for tile t lands just before its AV (lag-1)
                    vchunk(t)
                    if t % 4 == 3 and t < 12:
                        # next k column block, ahead of its score deadline
                        kq_chunk(wk, kTs, 0, t // 4 + 1)
                else:
                    budget[0] += ACT_STEP - STEP_COST
                    pop_fillers(blk_idx)
            emit_av(*pend[0])
            pend[0] = None
            for p in (1, 0):
                normalize(s, qb, p, (av0, av1)[p])

        def normalize(s, qb, p, av):
            sfx = f"{s}_{qb}_{p}"
            avu = avup.tile([DH + 1, QB], F32, tag="avu", name=f"avu{sfx}")
            nc.vector.tensor_copy(out=avu, in_=av)
            # spread the denominator over 64 partitions so the reciprocal runs
            # 64-wide instead of 8 cycles/elem on a single partition
            dsp = rcp.tile([DH, QB // DH], F32, tag="dsp", name=f"dsp{sfx}")
            nc.sync.dma_start(out=dsp, in_=avu[DH:DH + 1, :])
            rc = rcp.tile([DH, QB // DH], F32, tag="rc", name=f"rc{sfx}")
            nc.vector.reciprocal(out=rc, in_=dsp)
            dr = drbp.tile([QB], F32, tag="dr", name=f"dr{sfx}")
            dr_sq = bass.AP(tensor=dr.tensor, offset=dr.offset,
                            ap=[[QB // DH, DH], [1, QB // DH]])
            nc.sync.dma_start(out=dr_sq, in_=rc)
            dr_bc = bass.AP(tensor=dr.tensor, offset=dr.offset,
                            ap=[[0, DH]] + [list(dd) for dd in dr.ap])
            bcs = bcsp.tile([DH, QB], F32, tag="bcs", name=f"bcs{sfx}")
            nc.sync.dma_start(out=bcs, in_=dr_bc)
            if p == 0:
                with nc.allow_low_precision(reason="attn out in bf16"):
                    nc.vector.tensor_mul(
                        out=aoTs[s][0:DH, QB * qb:QB * (qb + 1)],
                        in0=avu[0:DH, :], in1=bcs)
            else:
                od = oddp.tile([DH, QB], BF16, tag="od", name=f"od{sfx}")
                with nc.allow_low_precision(reason="attn out in bf16"):
                    nc.vector.tensor_mul(out=od, in0=avu[0:DH, :], in1=bcs)
                nc.gpsimd.dma_start(
                    out=aoTs[s][DH:128, QB * qb:QB * (qb + 1)], in_=od)

        # ---- pre-phase: first k/q chunks for block (0,0) ----
        kq_chunk(wk, kTs, 0, 0)
        kq_chunk(wq, qTs, 0, 0)

        # ---- main loop: 16 blocks, s-major ----
        blk = 0
        for s in range(NPAIR):
            for qb in range(NQB):
                block(s, qb, blk, jit_v=(blk == 0))
                if s == NPAIR - 1:
                    # aoT rows for this qb now complete -> queue output proj
                    for ns in range(4 * qb, 4 * qb + 4):
                        for c in range(2):
                            fillers.append(
                                [1100, 98,
                                 (lambda ns=ns, c=c: oproj_half(ns, c))])
                blk += 1

        # ---- tail: drain remaining fillers (last oproj group) ----
        for cost, _, fn in fillers:
            fn()
        fillers.clear()


def get_program():
    if "nc" not in _CACHE:
        _CACHE["nc"] = _build_program()
    return _CACHE["nc"]


def make_in_maps(x, w_qkv, w_out, b_out):
    bf = ml_dtypes.bfloat16
    x = np.asarray(x, np.float32)
    w_qkv = np.asarray(w_qkv, np.float32)
    w_out = np.asarray(w_out, np.float32)
    b_out = np.asarray(b_out, np.float32)

    in_maps = []
    for core in range(NCORES):
        b, hh = core // 2, core % 2
        # xT in [128, KT, N] layout: [p, t, n] = x[b].T[t*128+p, n]
        xT = np.ascontiguousarray(x[b].T).astype(bf)                 # [DIM, N]
        xT_pt = np.ascontiguousarray(xT.reshape(KT, 128, N).transpose(1, 0, 2))
        # w slices for this head-half, groups ordered [v, k, q]
        wq = w_qkv[:, 512 * hh:512 * (hh + 1)]
        wk = w_qkv[:, DIM + 512 * hh:DIM + 512 * (hh + 1)]
        wv = w_qkv[:, 2 * DIM + 512 * hh:2 * DIM + 512 * (hh + 1)]
        wcat = np.stack([wv, wk, wq], axis=0).astype(bf)             # [3, DIM, 512]
        w_pt = np.ascontiguousarray(
            wcat.reshape(3, KT, 128, 512).transpose(2, 0, 1, 3))    # [p, g, t, e]
        # w_out rows for this half -> [p, hp, d]
        wo = w_out[512 * hh:512 * (hh + 1), :].astype(bf)            # [512, DIM]
        wo_pt = np.ascontiguousarray(wo.reshape(NPAIR, 128, DIM).transpose(1, 0, 2))
        in_maps.append({
            "xT": xT_pt,
            "w_qkv": w_pt,
            "w_out": wo_pt,
        })
    return in_maps


def kernel(x, w_qkv, w_out, b_out):
    nc = get_program()
    in_maps = make_in_maps(x, w_qkv, w_out, b_out)
    res = bass_utils.run_bass_kernel_spmd(nc, in_maps, core_ids=list(range(NCORES)))
    out = np.empty((B, N, DIM), np.float32)
    bias = np.asarray(b_out, np.float32)
    for b in range(B):
        out[b] = np.asarray(res.results[2 * b]["out"], np.float32)
        out[b] += np.asarray(res.results[2 * b + 1]["out"], np.float32)
        out[b] += bias
    return out
